# revision 27
# baseline (speedup 1.0000x reference)
"""Trainium2 Bass kernel for the MoE routing layer (nn_MoELayer_20358144983731).

Strategy
--------
Routing depends only on the atom's type (32 types), and with top-2-of-8
routing each atom needs exactly 3 expert MLPs (2 routed + 1 shared) instead
of the reference's dense 9.  The gate is tiny, so it is computed on the host;
atoms are sorted by type and packed into fixed-capacity slots (CAP=2048
atoms; types larger than CAP get a small spill slot), distributed across the
8 NeuronCores.  Every atom of a type shares the same two routed experts and
scalar routing weights, so the whole device program is data-driven (weights /
biases / scales arrive as per-core input tensors) and a single SPMD program
runs on all 8 cores.

Per slot the device computes, transposed (z.T = [dout, atoms]) so the
dout-dim bias lands on partitions:
    y = w0*tanh(X W0 + b0) + w1*tanh(X W1 + b1) + tanh(X Ws + bs)
Matmuls run in bf16 with fp32 PSUM accumulation (weights stationary, atoms
moving, N=512 per matmul, one 4-bank PSUM tile per stream, double
buffered); tanh+bias on the scalar engine (one op per stream); the 3-stream
combine is two fused scalar_tensor_tensor ops on the vector engine, split in
halves so output DMA starts early.
"""

import sys

import numpy as np

try:
    import concourse  # noqa: F401
except ImportError:  # grading container path
    sys.path.insert(0, "/opt/trn_rl_repo")

import ml_dtypes

import concourse.bacc as bacc
import concourse.mybir as mybir
import concourse.tile as tile
from concourse.bass_utils import run_bass_kernel_spmd

NB, NLOC = 4, 16384
DIN, DOUT = 256, 256
NTYPES = 32
N_CORES = 8
NS = 3  # streams: routed expert 0, routed expert 1, shared expert
CAP = 2048  # big-slot capacity (4 PSUM banks at fp32)
BF16 = ml_dtypes.bfloat16
WCOL = NS * 2 * 2 * 128  # weight columns per slot

_compiled_cache = {}


def _build_nc(nbig, nspill, sl):
    """Build + compile the SPMD Tile kernel.

    nbig:   number of CAP-length slots per core
    nspill: number of spill slots per core (0 = none)
    sl:     spill slot length (multiple of 128)
    """
    f32 = mybir.dt.float32
    bf16 = mybir.dt.bfloat16
    Tanh = mybir.ActivationFunctionType.Tanh
    mult = mybir.AluOpType.mult
    add = mybir.AluOpType.add

    nslots = nbig + nspill

    nc = bacc.Bacc("TRN2", target_bir_lowering=False, debug=False)
    xtb_d = nc.dram_tensor("xtb", [nbig * 128, 2 * CAP], bf16, kind="ExternalInput")
    if nspill:
        xts_d = nc.dram_tensor("xts", [nspill * 128, 2 * sl], bf16, kind="ExternalInput")
    w_d = nc.dram_tensor("w", [128, nslots * WCOL], bf16, kind="ExternalInput")
    b_d = nc.dram_tensor("b", [128, nslots * NS * 2], f32, kind="ExternalInput")
    s_d = nc.dram_tensor("s", [128, nslots * 2], f32, kind="ExternalInput")
    yb_d = nc.dram_tensor("yb", [nbig * 2 * 128, CAP], f32, kind="ExternalOutput")
    if nspill:
        ys_d = nc.dram_tensor("ys", [nspill * 2 * 128, sl], f32, kind="ExternalOutput")

    with tile.TileContext(nc) as tc:
        with (
            tc.tile_pool(name="const", bufs=1) as constp,
            tc.tile_pool(name="xt", bufs=3) as xtp,
            tc.tile_pool(name="t", bufs=2) as tp,
            tc.tile_pool(name="y", bufs=2) as yp,
            tc.tile_pool(name="ps", bufs=2, space="PSUM") as psp,
        ):
            # spill slots run first: their tiny DMAs land fast so PE starts
            # early, and the deep big-slot pipeline forms the kernel tail
            slot_order = list(range(nbig, nslots)) + list(range(nbig))
            first = slot_order[0]

            # first slot gets per-stream weight tiles so the first matmul only
            # waits on one 128KB transfer; other slots use one tile each
            w_first = [
                constp.tile([128, 512], bf16, name=f"wf{s}") for s in range(NS)
            ]
            w_sl = {
                si: constp.tile([128, WCOL], bf16, name=f"w{si}")
                for si in range(nslots)
                if si != first
            }
            # stream iteration order is (2, 0, 1) below
            nc.sync.dma_start(
                out=w_first[2],
                in_=w_d.ap()[:, first * WCOL + 1024 : first * WCOL + 1536],
            )
            xt0 = [
                xtp.tile([128, CAP if first < nbig else sl], bf16,
                         tag=f"xt{k}", name=f"xt{k}")
                for k in range(2)
            ]
            first_src = xtb_d if first < nbig else xts_d
            first_len = CAP if first < nbig else sl
            first_row = (first if first < nbig else first - nbig) * 128
            for k in range(2):
                nc.sync.dma_start(
                    out=xt0[k],
                    in_=first_src.ap()[
                        first_row : first_row + 128,
                        k * first_len : (k + 1) * first_len,
                    ],
                )
            # non-critical constants on the idle gpsimd SWDGE queue so issue
            # cost lands on neither the sync (xt/y) nor scalar (ACT) queues
            for s in (0, 1):
                nc.sync.dma_start(
                    out=w_first[s],
                    in_=w_d.ap()[
                        :, first * WCOL + s * 512 : first * WCOL + (s + 1) * 512
                    ],
                )
            b_sb = constp.tile([128, nslots * NS * 2], f32)
            nc.sync.dma_start(out=b_sb, in_=b_d.ap())
            s_sb = constp.tile([128, nslots * 2], f32)
            nc.sync.dma_start(out=s_sb, in_=s_d.ap())
            for si in slot_order[1:]:
                nc.sync.dma_start(
                    out=w_sl[si], in_=w_d.ap()[:, si * WCOL : (si + 1) * WCOL]
                )

            def issue_xt(si):
                big = si < nbig
                slen = CAP if big else sl
                src_d = xtb_d if big else xts_d
                row0 = (si if big else si - nbig) * 128
                tiles = [
                    xtp.tile([128, slen], bf16, tag=f"xt{k}", name=f"xt{k}")
                    for k in range(2)
                ]
                for k in range(2):
                    nc.sync.dma_start(
                        out=tiles[k],
                        in_=src_d.ap()[
                            row0 : row0 + 128, k * slen : (k + 1) * slen
                        ],
                    )
                return tiles

            xt_pending = {first: xt0}
            for nxt in slot_order[1:3]:
                xt_pending[nxt] = issue_xt(nxt)

            for oi, si in enumerate(slot_order):
                big = si < nbig
                slen = CAP if big else sl
                dst_d = yb_d if big else ys_d
                xt_sb = xt_pending.pop(si)
                if oi + 3 < len(slot_order):
                    xt_pending[slot_order[oi + 3]] = issue_xt(slot_order[oi + 3])
                for c in range(2):
                    t_sb = tp.tile([128, NS * slen], bf16, tag="t", name="t")
                    # shared stream (s=2) first: the combines need t2+t0 before
                    # t1, so the tail combine only waits on the last stream
                    for s in (2, 0, 1):
                        bcol = (si * NS + s) * 2 + c
                        ps = psp.tile([128, slen], f32, tag="ps", name="ps")
                        for k in range(2):
                            if si == first:
                                lhsT = w_first[s][:, (c * 2 + k) * 128 : (c * 2 + k + 1) * 128]
                            else:
                                blk = (s * 2 + c) * 2 + k
                                lhsT = w_sl[si][:, blk * 128 : (blk + 1) * 128]
                            for a0 in range(0, slen, 512):
                                al = min(512, slen - a0)
                                nc.tensor.matmul(
                                    ps[:, a0 : a0 + al],
                                    lhsT,
                                    xt_sb[k][:, a0 : a0 + al],
                                    start=(k == 0),
                                    stop=(k == 1),
                                )
                        # tanh + per-partition bias, PSUM -> SBUF (bf16)
                        nc.scalar.activation(
                            t_sb[:, s * slen : (s + 1) * slen],
                            ps,
                            Tanh,
                            bias=b_sb[:, bcol : bcol + 1],
                            scale=1.0,
                        )
                    yrow = ((si if big else si - nbig) * 2 + c) * 128
                    is_last = (oi == len(slot_order) - 1) and c == 1
                    if slen <= 512:
                        pieces = ((0, slen),)
                    elif is_last:
                        q = slen // 4
                        pieces = tuple((j * q, (j + 1) * q) for j in range(4))
                    else:
                        pieces = ((0, slen),)
                    ycomb = yp.tile([128, slen], f32, tag="yc", name="yc")
                    for h0, h1 in pieces:
                        ya = yp.tile([128, slen], f32, tag="ya", name="ya")
                        nc.vector.scalar_tensor_tensor(
                            ya[:, : h1 - h0],
                            t_sb[:, h0:h1],
                            s_sb[:, si * 2 : si * 2 + 1],
                            t_sb[:, 2 * slen + h0 : 2 * slen + h1],
                            mult,
                            add,
                        )
                        nc.vector.scalar_tensor_tensor(
                            ycomb[:, h0:h1],
                            t_sb[:, slen + h0 : slen + h1],
                            s_sb[:, si * 2 + 1 : si * 2 + 2],
                            ya[:, : h1 - h0],
                            mult,
                            add,
                        )
                        nc.gpsimd.dma_start(
                            out=dst_d.ap()[yrow : yrow + 128, h0:h1],
                            in_=ycomb[:, h0:h1],
                        )

    nc.compile()
    return nc


def _host_route(type_embeddings, gate_w):
    """Gate on host: per-type top-2 experts + softmax weights (tiny)."""
    logits = type_embeddings.astype(np.float32) @ gate_w.astype(np.float32)
    top2 = np.argsort(-logits, axis=1, kind="stable")[:, :2]
    tv = np.take_along_axis(logits, top2, axis=1)
    e = np.exp(tv - tv.max(axis=1, keepdims=True))
    wts = e / e.sum(axis=1, keepdims=True)
    return top2, wts


def _xt_layout(buf):
    """[nslots, slen, 256] fp32 -> [nslots*128, 2*slen] bf16 with
    row = slot*128 + p, col = k*slen + a, value = buf[slot, a, k*128+p]."""
    ns, slen, _ = buf.shape
    return np.ascontiguousarray(
        buf.reshape(ns, slen, 2, 128).transpose(0, 3, 2, 1)  # [slot, p, k, a]
    ).reshape(ns * 128, 2 * slen).astype(BF16)


def kernel(x, type_embeddings, atom_types, gate_w, expert_w, expert_b,
           shared_w, shared_b, _trace=False, _trace_kwargs=None):
    x = np.asarray(x, dtype=np.float32)
    type_embeddings = np.asarray(type_embeddings, dtype=np.float32)
    atom_types = np.asarray(atom_types)
    gate_w = np.asarray(gate_w, dtype=np.float32)
    expert_w = np.asarray(expert_w, dtype=np.float32)
    expert_b = np.asarray(expert_b, dtype=np.float32)
    shared_w = np.asarray(shared_w, dtype=np.float32)
    shared_b = np.asarray(shared_b, dtype=np.float32)

    top2, wts = _host_route(type_embeddings, gate_w)

    flat_t = atom_types.reshape(-1).astype(np.int64)
    N = flat_t.size
    order = np.argsort(flat_t, kind="stable")
    counts = np.bincount(flat_t, minlength=NTYPES)
    starts = np.zeros(NTYPES + 1, np.int64)
    starts[1:] = np.cumsum(counts)
    xs = x.reshape(N, DIN)[order]

    # pieces: per type a main piece (<= CAP rows) + spill pieces
    # big piece i of type t -> core t % N_CORES is NOT used; fixed layout:
    # big slot: type t -> core t // (NTYPES//N_CORES), slot t % (..)
    TPC = NTYPES // N_CORES  # big slots per core = 4
    spills = []  # (type, offset_in_type, length)
    for t in range(NTYPES):
        off = CAP
        while off < counts[t]:
            spills.append((t, off, min(CAP, counts[t] - off)))
            off += CAP
    nspill = (len(spills) + N_CORES - 1) // N_CORES
    max_spill = max((ln for _, _, ln in spills), default=0)
    sl = max(((max_spill + 127) // 128) * 128, 128) if nspill else 0

    big_buf = np.zeros((N_CORES, TPC, CAP, DIN), np.float32)
    for t in range(NTYPES):
        m = min(int(counts[t]), CAP)
        big_buf[t // TPC, t % TPC, :m] = xs[starts[t] : starts[t] + m]
    if nspill:
        sp_buf = np.zeros((N_CORES, nspill, sl, DIN), np.float32)
        sp_map = [[] for _ in range(N_CORES)]  # core -> [(slot, t, off, len)]
        for i, (t, off, ln) in enumerate(spills):
            core, slot = i % N_CORES, i // N_CORES
            sp_buf[core, slot, :ln] = xs[starts[t] + off : starts[t] + off + ln]
            sp_map[core].append((slot, t, off, ln))

    # per-(slot, stream) weight/bias/scale selection
    def slot_types(core):
        sts = [core * TPC + g for g in range(TPC)]
        if nspill:
            got = {slot: t for slot, t, _, _ in sp_map[core]}
            sts += [got.get(sidx, 0) for sidx in range(nspill)]
        return sts

    in_maps = []
    for core in range(N_CORES):
        sts = slot_types(core)
        nslots = len(sts)
        w_sel = np.empty((nslots, NS, DIN, DOUT), np.float32)
        b_sel = np.empty((nslots, NS, DOUT), np.float32)
        s_sel = np.empty((nslots, 2), np.float32)
        for i, t in enumerate(sts):
            e0, e1 = top2[t]
            w_sel[i, 0], w_sel[i, 1], w_sel[i, 2] = (
                expert_w[e0], expert_w[e1], shared_w[0],
            )
            b_sel[i, 0], b_sel[i, 1], b_sel[i, 2] = (
                expert_b[e0], expert_b[e1], shared_b[0],
            )
            s_sel[i] = wts[t]

        wb = (
            w_sel.reshape(nslots, NS, 2, 128, 2, 128)  # [i, s, k, p, c, q]
            .transpose(3, 0, 1, 4, 2, 5)  # [p, i, s, c, k, q]
            .reshape(128, nslots * WCOL)
            .astype(BF16)
        )
        bb = (
            b_sel.reshape(nslots, NS, 2, 128)  # [i, s, c, p]
            .transpose(3, 0, 1, 2)
            .reshape(128, nslots * NS * 2)
            .astype(np.float32)
        )
        sb_arr = np.broadcast_to(
            s_sel.reshape(1, nslots * 2), (128, nslots * 2)
        ).astype(np.float32)

        im = {
            "xtb": _xt_layout(big_buf[core]),
            "w": np.ascontiguousarray(wb),
            "b": np.ascontiguousarray(bb),
            "s": np.ascontiguousarray(sb_arr),
        }
        if nspill:
            im["xts"] = _xt_layout(sp_buf[core])
        in_maps.append(im)

    key = (TPC, nspill, sl)
    if key not in _compiled_cache:
        _compiled_cache[key] = _build_nc(TPC, nspill, sl)
    nc = _compiled_cache[key]

    kwargs = {}
    if _trace:
        kwargs["trace"] = True
        kwargs.update(_trace_kwargs or {})
    res = run_bass_kernel_spmd(nc, in_maps, core_ids=list(range(N_CORES)), **kwargs)

    # reassemble
    out_sorted = np.empty((N, DOUT), np.float32)
    for core in range(N_CORES):
        yb = res.results[core]["yb"].reshape(TPC, 2, 128, CAP)
        for g in range(TPC):
            t = core * TPC + g
            m = min(int(counts[t]), CAP)
            # [c, p, a] -> [a, c*128+p]
            blk = yb[g, :, :, :m].reshape(256, m).T
            out_sorted[starts[t] : starts[t] + m] = blk
        if nspill:
            ys = res.results[core]["ys"].reshape(nspill, 2, 128, sl)
            for slot, t, off, ln in sp_map[core]:
                blk = ys[slot, :, :, :ln].reshape(256, ln).T
                out_sorted[starts[t] + off : starts[t] + off + ln] = blk
    out = np.zeros((N, DOUT), np.float32)
    out[order] = out_sorted
    out = out.reshape(NB, NLOC, DOUT)

    if _trace:
        return out, res
    return out


# revision 28
# speedup vs baseline: 1.0304x; 1.0304x over previous
"""Trainium2 Bass kernel for the MoE routing layer (nn_MoELayer_20358144983731).

Strategy
--------
Routing depends only on the atom's type (32 types), and with top-2-of-8
routing each atom needs exactly 3 expert MLPs (2 routed + 1 shared) instead
of the reference's dense 9.  The gate is tiny, so it is computed on the host;
atoms are sorted by type and packed into fixed-capacity slots (CAP=2048
atoms; types larger than CAP get a small spill slot), distributed across the
8 NeuronCores.  Every atom of a type shares the same two routed experts and
scalar routing weights, so the whole device program is data-driven (weights /
biases / scales arrive as per-core input tensors) and a single SPMD program
runs on all 8 cores.

Per slot the device computes, transposed (z.T = [dout, atoms]) so the
dout-dim bias lands on partitions:
    y = w0*tanh(X W0 + b0) + w1*tanh(X W1 + b1) + tanh(X Ws + bs)
Matmuls run in bf16 with fp32 PSUM accumulation (weights stationary, atoms
moving, N=512 per matmul, one 4-bank PSUM tile per stream, double
buffered); tanh+bias on the scalar engine (one op per stream); the 3-stream
combine is two fused scalar_tensor_tensor ops on the vector engine, split in
halves so output DMA starts early.
"""

import sys

import numpy as np

try:
    import concourse  # noqa: F401
except ImportError:  # grading container path
    sys.path.insert(0, "/opt/trn_rl_repo")

import ml_dtypes

import concourse.bacc as bacc
import concourse.mybir as mybir
import concourse.tile as tile
from concourse.bass_utils import run_bass_kernel_spmd

NB, NLOC = 4, 16384
DIN, DOUT = 256, 256
NTYPES = 32
N_CORES = 8
NS = 3  # streams: routed expert 0, routed expert 1, shared expert
CAP = 2048  # big-slot capacity (4 PSUM banks at fp32)
BF16 = ml_dtypes.bfloat16
WCOL = NS * 2 * 2 * 128  # weight columns per slot

_compiled_cache = {}


def _build_nc(nbig, nspill, sl):
    """Build + compile the SPMD Tile kernel.

    nbig:   number of CAP-length slots per core
    nspill: number of spill slots per core (0 = none)
    sl:     spill slot length (multiple of 128)
    """
    f32 = mybir.dt.float32
    bf16 = mybir.dt.bfloat16
    Tanh = mybir.ActivationFunctionType.Tanh
    mult = mybir.AluOpType.mult
    add = mybir.AluOpType.add

    nslots = nbig + nspill

    nc = bacc.Bacc("TRN2", target_bir_lowering=False, debug=False)
    xtb_d = nc.dram_tensor("xtb", [nbig * 128, 2 * CAP], bf16, kind="ExternalInput")
    if nspill:
        xts_d = nc.dram_tensor("xts", [nspill * 128, 2 * sl], bf16, kind="ExternalInput")
    w_d = nc.dram_tensor("w", [128, nslots * WCOL], bf16, kind="ExternalInput")
    b_d = nc.dram_tensor("b", [128, nslots * NS * 2], f32, kind="ExternalInput")
    s_d = nc.dram_tensor("s", [128, nslots * 2], f32, kind="ExternalInput")
    yb_d = nc.dram_tensor("yb", [nbig * 2 * 128, CAP], f32, kind="ExternalOutput")
    if nspill:
        ys_d = nc.dram_tensor("ys", [nspill * 2 * 128, sl], f32, kind="ExternalOutput")

    with tile.TileContext(nc) as tc:
        with (
            tc.tile_pool(name="const", bufs=1) as constp,
            tc.tile_pool(name="xt", bufs=3) as xtp,
            tc.tile_pool(name="t", bufs=2) as tp,
            tc.tile_pool(name="y", bufs=2) as yp,
            tc.tile_pool(name="ps", bufs=2, space="PSUM") as psp,
        ):
            # spill slots run first: their tiny DMAs land fast so PE starts
            # early, and the deep big-slot pipeline forms the kernel tail
            slot_order = list(range(nbig, nslots)) + list(range(nbig))
            first = slot_order[0]

            # first slot gets per-stream weight tiles so the first matmul only
            # waits on one 128KB transfer; other slots use one tile each
            w_first = [
                constp.tile([128, 512], bf16, name=f"wf{s}") for s in range(NS)
            ]
            w_sl = {
                si: constp.tile([128, WCOL], bf16, name=f"w{si}")
                for si in range(nslots)
                if si != first
            }
            # stream iteration order is (2, 0, 1) below
            nc.sync.dma_start(
                out=w_first[2],
                in_=w_d.ap()[:, first * WCOL + 1024 : first * WCOL + 1536],
            )
            xt0 = [
                xtp.tile([128, CAP if first < nbig else sl], bf16,
                         tag=f"xt{k}", name=f"xt{k}")
                for k in range(2)
            ]
            first_src = xtb_d if first < nbig else xts_d
            first_len = CAP if first < nbig else sl
            first_row = (first if first < nbig else first - nbig) * 128
            for k in range(2):
                nc.sync.dma_start(
                    out=xt0[k],
                    in_=first_src.ap()[
                        first_row : first_row + 128,
                        k * first_len : (k + 1) * first_len,
                    ],
                )
            # non-critical constants on the idle gpsimd SWDGE queue so issue
            # cost lands on neither the sync (xt/y) nor scalar (ACT) queues
            for s in (0, 1):
                nc.sync.dma_start(
                    out=w_first[s],
                    in_=w_d.ap()[
                        :, first * WCOL + s * 512 : first * WCOL + (s + 1) * 512
                    ],
                )
            b_sb = constp.tile([128, nslots * NS * 2], f32)
            nc.sync.dma_start(out=b_sb, in_=b_d.ap())
            s_sb = constp.tile([128, nslots * 2], f32)
            nc.sync.dma_start(out=s_sb, in_=s_d.ap())
            for si in slot_order[1:]:
                nc.sync.dma_start(
                    out=w_sl[si], in_=w_d.ap()[:, si * WCOL : (si + 1) * WCOL]
                )

            def issue_xt(si):
                big = si < nbig
                slen = CAP if big else sl
                src_d = xtb_d if big else xts_d
                row0 = (si if big else si - nbig) * 128
                tiles = [
                    xtp.tile([128, slen], bf16, tag=f"xt{k}", name=f"xt{k}")
                    for k in range(2)
                ]
                for k in range(2):
                    nc.sync.dma_start(
                        out=tiles[k],
                        in_=src_d.ap()[
                            row0 : row0 + 128, k * slen : (k + 1) * slen
                        ],
                    )
                return tiles

            xt_pending = {first: xt0}
            for nxt in slot_order[1:3]:
                xt_pending[nxt] = issue_xt(nxt)

            for oi, si in enumerate(slot_order):
                big = si < nbig
                slen = CAP if big else sl
                dst_d = yb_d if big else ys_d
                xt_sb = xt_pending.pop(si)
                if oi + 3 < len(slot_order):
                    xt_pending[slot_order[oi + 3]] = issue_xt(slot_order[oi + 3])
                for c in range(2):
                    t_sb = tp.tile([128, NS * slen], bf16, tag="t", name="t")
                    # shared stream (s=2) first: the combines need t2+t0 before
                    # t1, so the tail combine only waits on the last stream
                    for s in (2, 0, 1):
                        bcol = (si * NS + s) * 2 + c
                        ps = psp.tile([128, slen], f32, tag="ps", name="ps")
                        for k in range(2):
                            if si == first:
                                lhsT = w_first[s][:, (c * 2 + k) * 128 : (c * 2 + k + 1) * 128]
                            else:
                                blk = (s * 2 + c) * 2 + k
                                lhsT = w_sl[si][:, blk * 128 : (blk + 1) * 128]
                            for a0 in range(0, slen, 512):
                                al = min(512, slen - a0)
                                nc.tensor.matmul(
                                    ps[:, a0 : a0 + al],
                                    lhsT,
                                    xt_sb[k][:, a0 : a0 + al],
                                    start=(k == 0),
                                    stop=(k == 1),
                                )
                        # tanh + per-partition bias, PSUM -> SBUF (bf16)
                        nc.scalar.activation(
                            t_sb[:, s * slen : (s + 1) * slen],
                            ps,
                            Tanh,
                            bias=b_sb[:, bcol : bcol + 1],
                            scale=1.0,
                        )
                    yrow = ((si if big else si - nbig) * 2 + c) * 128
                    is_last = (oi == len(slot_order) - 1) and c == 1
                    if slen <= 512:
                        pieces = ((0, slen),)
                    elif is_last:
                        q = slen // 4
                        pieces = tuple((j * q, (j + 1) * q) for j in range(4))
                    else:
                        pieces = ((0, slen),)
                    ycomb = yp.tile([128, slen], f32, tag="yc", name="yc")
                    for h0, h1 in pieces:
                        ya = yp.tile([128, slen], f32, tag="ya", name="ya")
                        nc.vector.scalar_tensor_tensor(
                            ya[:, : h1 - h0],
                            t_sb[:, h0:h1],
                            s_sb[:, si * 2 : si * 2 + 1],
                            t_sb[:, 2 * slen + h0 : 2 * slen + h1],
                            mult,
                            add,
                        )
                        nc.vector.scalar_tensor_tensor(
                            ycomb[:, h0:h1],
                            t_sb[:, slen + h0 : slen + h1],
                            s_sb[:, si * 2 + 1 : si * 2 + 2],
                            ya[:, : h1 - h0],
                            mult,
                            add,
                        )
                        nc.sync.dma_start(
                            out=dst_d.ap()[yrow : yrow + 128, h0:h1],
                            in_=ycomb[:, h0:h1],
                        )

    nc.compile()
    return nc


def _host_route(type_embeddings, gate_w):
    """Gate on host: per-type top-2 experts + softmax weights (tiny)."""
    logits = type_embeddings.astype(np.float32) @ gate_w.astype(np.float32)
    top2 = np.argsort(-logits, axis=1, kind="stable")[:, :2]
    tv = np.take_along_axis(logits, top2, axis=1)
    e = np.exp(tv - tv.max(axis=1, keepdims=True))
    wts = e / e.sum(axis=1, keepdims=True)
    return top2, wts


def _xt_layout(buf):
    """[nslots, slen, 256] fp32 -> [nslots*128, 2*slen] bf16 with
    row = slot*128 + p, col = k*slen + a, value = buf[slot, a, k*128+p]."""
    ns, slen, _ = buf.shape
    return np.ascontiguousarray(
        buf.reshape(ns, slen, 2, 128).transpose(0, 3, 2, 1)  # [slot, p, k, a]
    ).reshape(ns * 128, 2 * slen).astype(BF16)


def kernel(x, type_embeddings, atom_types, gate_w, expert_w, expert_b,
           shared_w, shared_b, _trace=False, _trace_kwargs=None):
    x = np.asarray(x, dtype=np.float32)
    type_embeddings = np.asarray(type_embeddings, dtype=np.float32)
    atom_types = np.asarray(atom_types)
    gate_w = np.asarray(gate_w, dtype=np.float32)
    expert_w = np.asarray(expert_w, dtype=np.float32)
    expert_b = np.asarray(expert_b, dtype=np.float32)
    shared_w = np.asarray(shared_w, dtype=np.float32)
    shared_b = np.asarray(shared_b, dtype=np.float32)

    top2, wts = _host_route(type_embeddings, gate_w)

    flat_t = atom_types.reshape(-1).astype(np.int64)
    N = flat_t.size
    order = np.argsort(flat_t, kind="stable")
    counts = np.bincount(flat_t, minlength=NTYPES)
    starts = np.zeros(NTYPES + 1, np.int64)
    starts[1:] = np.cumsum(counts)
    xs = x.reshape(N, DIN)[order]

    # pieces: per type a main piece (<= CAP rows) + spill pieces
    # big piece i of type t -> core t % N_CORES is NOT used; fixed layout:
    # big slot: type t -> core t // (NTYPES//N_CORES), slot t % (..)
    TPC = NTYPES // N_CORES  # big slots per core = 4
    spills = []  # (type, offset_in_type, length)
    for t in range(NTYPES):
        off = CAP
        while off < counts[t]:
            spills.append((t, off, min(CAP, counts[t] - off)))
            off += CAP
    nspill = (len(spills) + N_CORES - 1) // N_CORES
    max_spill = max((ln for _, _, ln in spills), default=0)
    sl = max(((max_spill + 127) // 128) * 128, 128) if nspill else 0

    big_buf = np.zeros((N_CORES, TPC, CAP, DIN), np.float32)
    for t in range(NTYPES):
        m = min(int(counts[t]), CAP)
        big_buf[t // TPC, t % TPC, :m] = xs[starts[t] : starts[t] + m]
    if nspill:
        sp_buf = np.zeros((N_CORES, nspill, sl, DIN), np.float32)
        sp_map = [[] for _ in range(N_CORES)]  # core -> [(slot, t, off, len)]
        for i, (t, off, ln) in enumerate(spills):
            core, slot = i % N_CORES, i // N_CORES
            sp_buf[core, slot, :ln] = xs[starts[t] + off : starts[t] + off + ln]
            sp_map[core].append((slot, t, off, ln))

    # per-(slot, stream) weight/bias/scale selection
    def slot_types(core):
        sts = [core * TPC + g for g in range(TPC)]
        if nspill:
            got = {slot: t for slot, t, _, _ in sp_map[core]}
            sts += [got.get(sidx, 0) for sidx in range(nspill)]
        return sts

    in_maps = []
    for core in range(N_CORES):
        sts = slot_types(core)
        nslots = len(sts)
        w_sel = np.empty((nslots, NS, DIN, DOUT), np.float32)
        b_sel = np.empty((nslots, NS, DOUT), np.float32)
        s_sel = np.empty((nslots, 2), np.float32)
        for i, t in enumerate(sts):
            e0, e1 = top2[t]
            w_sel[i, 0], w_sel[i, 1], w_sel[i, 2] = (
                expert_w[e0], expert_w[e1], shared_w[0],
            )
            b_sel[i, 0], b_sel[i, 1], b_sel[i, 2] = (
                expert_b[e0], expert_b[e1], shared_b[0],
            )
            s_sel[i] = wts[t]

        wb = (
            w_sel.reshape(nslots, NS, 2, 128, 2, 128)  # [i, s, k, p, c, q]
            .transpose(3, 0, 1, 4, 2, 5)  # [p, i, s, c, k, q]
            .reshape(128, nslots * WCOL)
            .astype(BF16)
        )
        bb = (
            b_sel.reshape(nslots, NS, 2, 128)  # [i, s, c, p]
            .transpose(3, 0, 1, 2)
            .reshape(128, nslots * NS * 2)
            .astype(np.float32)
        )
        sb_arr = np.broadcast_to(
            s_sel.reshape(1, nslots * 2), (128, nslots * 2)
        ).astype(np.float32)

        im = {
            "xtb": _xt_layout(big_buf[core]),
            "w": np.ascontiguousarray(wb),
            "b": np.ascontiguousarray(bb),
            "s": np.ascontiguousarray(sb_arr),
        }
        if nspill:
            im["xts"] = _xt_layout(sp_buf[core])
        in_maps.append(im)

    key = (TPC, nspill, sl)
    if key not in _compiled_cache:
        _compiled_cache[key] = _build_nc(TPC, nspill, sl)
    nc = _compiled_cache[key]

    kwargs = {}
    if _trace:
        kwargs["trace"] = True
        kwargs.update(_trace_kwargs or {})
    res = run_bass_kernel_spmd(nc, in_maps, core_ids=list(range(N_CORES)), **kwargs)

    # reassemble
    out_sorted = np.empty((N, DOUT), np.float32)
    for core in range(N_CORES):
        yb = res.results[core]["yb"].reshape(TPC, 2, 128, CAP)
        for g in range(TPC):
            t = core * TPC + g
            m = min(int(counts[t]), CAP)
            # [c, p, a] -> [a, c*128+p]
            blk = yb[g, :, :, :m].reshape(256, m).T
            out_sorted[starts[t] : starts[t] + m] = blk
        if nspill:
            ys = res.results[core]["ys"].reshape(nspill, 2, 128, sl)
            for slot, t, off, ln in sp_map[core]:
                blk = ys[slot, :, :, :ln].reshape(256, ln).T
                out_sorted[starts[t] + off : starts[t] + off + ln] = blk
    out = np.zeros((N, DOUT), np.float32)
    out[order] = out_sorted
    out = out.reshape(NB, NLOC, DOUT)

    if _trace:
        return out, res
    return out


# revision 29
# speedup vs baseline: 1.1196x; 1.0866x over previous
"""Trainium2 Bass kernel for the MoE routing layer (nn_MoELayer_20358144983731).

Strategy
--------
Routing depends only on the atom's type (32 types), and with top-2-of-8
routing each atom needs exactly 3 expert MLPs (2 routed + 1 shared) instead
of the reference's dense 9.  The gate is tiny, so it is computed on the host;
atoms are sorted by type and packed into fixed-capacity slots (CAP=2048
atoms; types larger than CAP get a small spill slot), distributed across the
8 NeuronCores.  Every atom of a type shares the same two routed experts and
scalar routing weights, so the whole device program is data-driven (weights /
biases / scales arrive as per-core input tensors) and a single SPMD program
runs on all 8 cores.

Per slot the device computes, transposed (z.T = [dout, atoms]) so the
dout-dim bias lands on partitions:
    y = w0*tanh(X W0 + b0) + w1*tanh(X W1 + b1) + tanh(X Ws + bs)
Matmuls run in bf16 with fp32 PSUM accumulation (weights stationary, atoms
moving, N=512 per matmul, one 4-bank PSUM tile per stream, double
buffered); tanh+bias on the scalar engine (one op per stream); the 3-stream
combine is two fused scalar_tensor_tensor ops on the vector engine, split in
halves so output DMA starts early.
"""

import sys

import numpy as np

try:
    import concourse  # noqa: F401
except ImportError:  # grading container path
    sys.path.insert(0, "/opt/trn_rl_repo")

import ml_dtypes

import concourse.bacc as bacc
import concourse.mybir as mybir
import concourse.tile as tile
from concourse.bass_utils import run_bass_kernel_spmd

NB, NLOC = 4, 16384
DIN, DOUT = 256, 256
NTYPES = 32
N_CORES = 8
NS = 3  # streams: routed expert 0, routed expert 1, shared expert
CAP = 2048  # big-slot capacity (4 PSUM banks at fp32)
BF16 = ml_dtypes.bfloat16
WCOL = NS * 2 * 2 * 128  # weight columns per slot

_compiled_cache = {}


def _build_nc(nbig, nspill, sl):
    """Build + compile the SPMD Tile kernel.

    nbig:   number of CAP-length slots per core
    nspill: number of spill slots per core (0 = none)
    sl:     spill slot length (multiple of 128)
    """
    f32 = mybir.dt.float32
    bf16 = mybir.dt.bfloat16
    Tanh = mybir.ActivationFunctionType.Tanh
    mult = mybir.AluOpType.mult
    add = mybir.AluOpType.add

    nslots = nbig + nspill

    nc = bacc.Bacc("TRN2", target_bir_lowering=False, debug=False)
    xtb_d = nc.dram_tensor("xtb", [nbig * 128, 2 * CAP], bf16, kind="ExternalInput")
    if nspill:
        xts_d = nc.dram_tensor("xts", [nspill * 128, 2 * sl], bf16, kind="ExternalInput")
    w_d = nc.dram_tensor("w", [128, nslots * WCOL], bf16, kind="ExternalInput")
    b_d = nc.dram_tensor("b", [128, nslots * NS * 2], f32, kind="ExternalInput")
    s_d = nc.dram_tensor("s", [128, nslots * 2], f32, kind="ExternalInput")
    yb_d = nc.dram_tensor("yb", [nbig * 2 * 128, CAP], f32, kind="ExternalOutput")
    if nspill:
        ys_d = nc.dram_tensor("ys", [nspill * 2 * 128, sl], f32, kind="ExternalOutput")

    with tile.TileContext(nc) as tc:
        with (
            tc.tile_pool(name="const", bufs=1) as constp,
            tc.tile_pool(name="xt", bufs=3) as xtp,
            tc.tile_pool(name="t", bufs=2) as tp,
            tc.tile_pool(name="y", bufs=2) as yp,
            tc.tile_pool(name="ps", bufs=2, space="PSUM") as psp,
        ):
            # spill slots run first: their tiny DMAs land fast so PE starts
            # early, and the deep big-slot pipeline forms the kernel tail
            slot_order = list(range(nbig, nslots)) + list(range(nbig))
            first = slot_order[0]

            # first slot gets per-stream weight tiles so the first matmul only
            # waits on one 128KB transfer; other slots use one tile each
            w_first = [
                constp.tile([128, 512], bf16, name=f"wf{s}") for s in range(NS)
            ]
            w_sl = {
                si: constp.tile([128, WCOL], bf16, name=f"w{si}")
                for si in range(nslots)
                if si != first
            }
            # stream iteration order is (2, 0, 1) below
            nc.sync.dma_start(
                out=w_first[2],
                in_=w_d.ap()[:, first * WCOL + 1024 : first * WCOL + 1536],
            )
            xt0 = [
                xtp.tile([128, CAP if first < nbig else sl], bf16,
                         tag=f"xt{k}", name=f"xt{k}")
                for k in range(2)
            ]
            first_src = xtb_d if first < nbig else xts_d
            first_len = CAP if first < nbig else sl
            first_row = (first if first < nbig else first - nbig) * 128
            for k in range(2):
                nc.sync.dma_start(
                    out=xt0[k],
                    in_=first_src.ap()[
                        first_row : first_row + 128,
                        k * first_len : (k + 1) * first_len,
                    ],
                )
            # non-critical constants on the idle gpsimd SWDGE queue so issue
            # cost lands on neither the sync (xt/y) nor scalar (ACT) queues
            for s in (0, 1):
                nc.scalar.dma_start(
                    out=w_first[s],
                    in_=w_d.ap()[
                        :, first * WCOL + s * 512 : first * WCOL + (s + 1) * 512
                    ],
                )
            b_sb = constp.tile([128, nslots * NS * 2], f32)
            nc.scalar.dma_start(out=b_sb, in_=b_d.ap())
            s_sb = constp.tile([128, nslots * 2], f32)
            nc.scalar.dma_start(out=s_sb, in_=s_d.ap())
            for si in slot_order[1:]:
                nc.scalar.dma_start(
                    out=w_sl[si], in_=w_d.ap()[:, si * WCOL : (si + 1) * WCOL]
                )

            def issue_xt(si):
                big = si < nbig
                slen = CAP if big else sl
                src_d = xtb_d if big else xts_d
                row0 = (si if big else si - nbig) * 128
                tiles = [
                    xtp.tile([128, slen], bf16, tag=f"xt{k}", name=f"xt{k}")
                    for k in range(2)
                ]
                for k in range(2):
                    nc.sync.dma_start(
                        out=tiles[k],
                        in_=src_d.ap()[
                            row0 : row0 + 128, k * slen : (k + 1) * slen
                        ],
                    )
                return tiles

            xt_pending = {first: xt0}
            for nxt in slot_order[1:3]:
                xt_pending[nxt] = issue_xt(nxt)

            for oi, si in enumerate(slot_order):
                big = si < nbig
                slen = CAP if big else sl
                dst_d = yb_d if big else ys_d
                xt_sb = xt_pending.pop(si)
                if oi + 3 < len(slot_order):
                    xt_pending[slot_order[oi + 3]] = issue_xt(slot_order[oi + 3])
                for c in range(2):
                    t_sb = tp.tile([128, NS * slen], bf16, tag="t", name="t")
                    # shared stream (s=2) first: the combines need t2+t0 before
                    # t1, so the tail combine only waits on the last stream
                    for s in (2, 0, 1):
                        bcol = (si * NS + s) * 2 + c
                        ps = psp.tile([128, slen], f32, tag="ps", name="ps")
                        for k in range(2):
                            if si == first:
                                lhsT = w_first[s][:, (c * 2 + k) * 128 : (c * 2 + k + 1) * 128]
                            else:
                                blk = (s * 2 + c) * 2 + k
                                lhsT = w_sl[si][:, blk * 128 : (blk + 1) * 128]
                            for a0 in range(0, slen, 512):
                                al = min(512, slen - a0)
                                nc.tensor.matmul(
                                    ps[:, a0 : a0 + al],
                                    lhsT,
                                    xt_sb[k][:, a0 : a0 + al],
                                    start=(k == 0),
                                    stop=(k == 1),
                                )
                        # tanh + per-partition bias, PSUM -> SBUF (bf16)
                        nc.scalar.activation(
                            t_sb[:, s * slen : (s + 1) * slen],
                            ps,
                            Tanh,
                            bias=b_sb[:, bcol : bcol + 1],
                            scale=1.0,
                        )
                    yrow = ((si if big else si - nbig) * 2 + c) * 128
                    is_last = (oi == len(slot_order) - 1) and c == 1
                    if slen <= 512:
                        pieces = ((0, slen),)
                    elif is_last:
                        q = slen // 4
                        pieces = tuple((j * q, (j + 1) * q) for j in range(4))
                    else:
                        pieces = ((0, slen),)
                    ycomb = yp.tile([128, slen], f32, tag="yc", name="yc")
                    for h0, h1 in pieces:
                        ya = yp.tile([128, slen], f32, tag="ya", name="ya")
                        nc.vector.scalar_tensor_tensor(
                            ya[:, : h1 - h0],
                            t_sb[:, h0:h1],
                            s_sb[:, si * 2 : si * 2 + 1],
                            t_sb[:, 2 * slen + h0 : 2 * slen + h1],
                            mult,
                            add,
                        )
                        nc.vector.scalar_tensor_tensor(
                            ycomb[:, h0:h1],
                            t_sb[:, slen + h0 : slen + h1],
                            s_sb[:, si * 2 + 1 : si * 2 + 2],
                            ya[:, : h1 - h0],
                            mult,
                            add,
                        )
                        nc.sync.dma_start(
                            out=dst_d.ap()[yrow : yrow + 128, h0:h1],
                            in_=ycomb[:, h0:h1],
                        )

    nc.compile()
    return nc


def _host_route(type_embeddings, gate_w):
    """Gate on host: per-type top-2 experts + softmax weights (tiny)."""
    logits = type_embeddings.astype(np.float32) @ gate_w.astype(np.float32)
    top2 = np.argsort(-logits, axis=1, kind="stable")[:, :2]
    tv = np.take_along_axis(logits, top2, axis=1)
    e = np.exp(tv - tv.max(axis=1, keepdims=True))
    wts = e / e.sum(axis=1, keepdims=True)
    return top2, wts


def _xt_layout(buf):
    """[nslots, slen, 256] fp32 -> [nslots*128, 2*slen] bf16 with
    row = slot*128 + p, col = k*slen + a, value = buf[slot, a, k*128+p]."""
    ns, slen, _ = buf.shape
    return np.ascontiguousarray(
        buf.reshape(ns, slen, 2, 128).transpose(0, 3, 2, 1)  # [slot, p, k, a]
    ).reshape(ns * 128, 2 * slen).astype(BF16)


def kernel(x, type_embeddings, atom_types, gate_w, expert_w, expert_b,
           shared_w, shared_b, _trace=False, _trace_kwargs=None):
    x = np.asarray(x, dtype=np.float32)
    type_embeddings = np.asarray(type_embeddings, dtype=np.float32)
    atom_types = np.asarray(atom_types)
    gate_w = np.asarray(gate_w, dtype=np.float32)
    expert_w = np.asarray(expert_w, dtype=np.float32)
    expert_b = np.asarray(expert_b, dtype=np.float32)
    shared_w = np.asarray(shared_w, dtype=np.float32)
    shared_b = np.asarray(shared_b, dtype=np.float32)

    top2, wts = _host_route(type_embeddings, gate_w)

    flat_t = atom_types.reshape(-1).astype(np.int64)
    N = flat_t.size
    order = np.argsort(flat_t, kind="stable")
    counts = np.bincount(flat_t, minlength=NTYPES)
    starts = np.zeros(NTYPES + 1, np.int64)
    starts[1:] = np.cumsum(counts)
    xs = x.reshape(N, DIN)[order]

    # pieces: per type a main piece (<= CAP rows) + spill pieces
    # big piece i of type t -> core t % N_CORES is NOT used; fixed layout:
    # big slot: type t -> core t // (NTYPES//N_CORES), slot t % (..)
    TPC = NTYPES // N_CORES  # big slots per core = 4
    spills = []  # (type, offset_in_type, length)
    for t in range(NTYPES):
        off = CAP
        while off < counts[t]:
            spills.append((t, off, min(CAP, counts[t] - off)))
            off += CAP
    nspill = (len(spills) + N_CORES - 1) // N_CORES
    max_spill = max((ln for _, _, ln in spills), default=0)
    sl = max(((max_spill + 127) // 128) * 128, 128) if nspill else 0

    big_buf = np.zeros((N_CORES, TPC, CAP, DIN), np.float32)
    for t in range(NTYPES):
        m = min(int(counts[t]), CAP)
        big_buf[t // TPC, t % TPC, :m] = xs[starts[t] : starts[t] + m]
    if nspill:
        sp_buf = np.zeros((N_CORES, nspill, sl, DIN), np.float32)
        sp_map = [[] for _ in range(N_CORES)]  # core -> [(slot, t, off, len)]
        for i, (t, off, ln) in enumerate(spills):
            core, slot = i % N_CORES, i // N_CORES
            sp_buf[core, slot, :ln] = xs[starts[t] + off : starts[t] + off + ln]
            sp_map[core].append((slot, t, off, ln))

    # per-(slot, stream) weight/bias/scale selection
    def slot_types(core):
        sts = [core * TPC + g for g in range(TPC)]
        if nspill:
            got = {slot: t for slot, t, _, _ in sp_map[core]}
            sts += [got.get(sidx, 0) for sidx in range(nspill)]
        return sts

    in_maps = []
    for core in range(N_CORES):
        sts = slot_types(core)
        nslots = len(sts)
        w_sel = np.empty((nslots, NS, DIN, DOUT), np.float32)
        b_sel = np.empty((nslots, NS, DOUT), np.float32)
        s_sel = np.empty((nslots, 2), np.float32)
        for i, t in enumerate(sts):
            e0, e1 = top2[t]
            w_sel[i, 0], w_sel[i, 1], w_sel[i, 2] = (
                expert_w[e0], expert_w[e1], shared_w[0],
            )
            b_sel[i, 0], b_sel[i, 1], b_sel[i, 2] = (
                expert_b[e0], expert_b[e1], shared_b[0],
            )
            s_sel[i] = wts[t]

        wb = (
            w_sel.reshape(nslots, NS, 2, 128, 2, 128)  # [i, s, k, p, c, q]
            .transpose(3, 0, 1, 4, 2, 5)  # [p, i, s, c, k, q]
            .reshape(128, nslots * WCOL)
            .astype(BF16)
        )
        bb = (
            b_sel.reshape(nslots, NS, 2, 128)  # [i, s, c, p]
            .transpose(3, 0, 1, 2)
            .reshape(128, nslots * NS * 2)
            .astype(np.float32)
        )
        sb_arr = np.broadcast_to(
            s_sel.reshape(1, nslots * 2), (128, nslots * 2)
        ).astype(np.float32)

        im = {
            "xtb": _xt_layout(big_buf[core]),
            "w": np.ascontiguousarray(wb),
            "b": np.ascontiguousarray(bb),
            "s": np.ascontiguousarray(sb_arr),
        }
        if nspill:
            im["xts"] = _xt_layout(sp_buf[core])
        in_maps.append(im)

    key = (TPC, nspill, sl)
    if key not in _compiled_cache:
        _compiled_cache[key] = _build_nc(TPC, nspill, sl)
    nc = _compiled_cache[key]

    kwargs = {}
    if _trace:
        kwargs["trace"] = True
        kwargs.update(_trace_kwargs or {})
    res = run_bass_kernel_spmd(nc, in_maps, core_ids=list(range(N_CORES)), **kwargs)

    # reassemble
    out_sorted = np.empty((N, DOUT), np.float32)
    for core in range(N_CORES):
        yb = res.results[core]["yb"].reshape(TPC, 2, 128, CAP)
        for g in range(TPC):
            t = core * TPC + g
            m = min(int(counts[t]), CAP)
            # [c, p, a] -> [a, c*128+p]
            blk = yb[g, :, :, :m].reshape(256, m).T
            out_sorted[starts[t] : starts[t] + m] = blk
        if nspill:
            ys = res.results[core]["ys"].reshape(nspill, 2, 128, sl)
            for slot, t, off, ln in sp_map[core]:
                blk = ys[slot, :, :, :ln].reshape(256, ln).T
                out_sorted[starts[t] + off : starts[t] + off + ln] = blk
    out = np.zeros((N, DOUT), np.float32)
    out[order] = out_sorted
    out = out.reshape(NB, NLOC, DOUT)

    if _trace:
        return out, res
    return out
